# revision 8
# baseline (speedup 1.0000x reference)
"""Trainium2 Bass kernel for CenterWoParamMultiCosineLossV2.

Math (per sample b with label l):
    d_k   = 1 + <x_b, centers[l, k]>          k = 0..7
    value = (sum_k d_k^2) / (sum_k d_k)
    loss  = mean_b value
With u = sum_k <x_b, c_k> = <x_b, csum_l> and q = sum_k <x_b, c_k>^2:
    den = 8 + u,  num = 8 + 2u + q,  value = num / den

Only u needs fp32-grade precision (near-singular denominators: the
min |den| sample amplifies u error ~12000x into the loss); q tolerates
~0.1% error.  fp32r/bf16 matmuls round the moving x operand to ~11
mantissa bits (measured: u error 3.3e-2), so the main matmul must be
fp32 (LOW_HIGH, measured u error 4e-5).

Strategy (loss is a mean -> permutation invariant):
  * Host sorts samples by label; each of 8 cores takes 1024 consecutive
    sorted samples (~13 classes).  Per core, a class table [512, 128]
    holds, for each local class slot j: its 8 centers at columns
    8j..8j+7 and its center-sum at column M8+j  (M8 = 8*nslot).
  * One fp32 PE matmul chain per 512-sample half: S = table^T @ x^T in
    PSUM [128, 512]; rows 0..M8 are per-center scores s, rows M8..M9
    are u candidates per slot.
  * A tiny bf16 matmul expands the per-sample one-hot e [nslot, 512]
    through a fixed pattern G into a row mask [M9, 512] (1 where the
    row belongs to the sample's slot).
  * DVE: tmp = S * mask;  t[0:M8] = tmp^2 (= s^2 masked);  ACT copies
    t[M8:M9] = tmp (masked u).  Row M9 of t is constant 1.0.
  * One fp32 reduce matmul with stationary sel [M9+1, 2] computes
    num = 1*sum(s^2) + 2*sum(u) + 8  and  den = sum(u) + 8 directly
    (weights 1/2/8 in the stationary).  DVE reciprocal + multiply give
    value per sample; one 4 KiB DMA returns [2, 512] values per core.
  * Host sums the 8192 values and divides by 8192.
"""

import numpy as np
from contextlib import ExitStack

import concourse.bass as bass
import concourse.tile as tile
import concourse.mybir as mybir
from concourse import bass_utils
from concourse.masks import make_identity

# ---------------------------------------------------------------------------
# Workaround: this walrus build accepts only ONE sem-wait per instruction
# ("Too many sync wait commands"), but Tile freely attaches several waits at
# join points.  Post-pass: for any instruction with k>1 waits, hoist k-1 of
# them onto same-engine nops inserted immediately before it.  Tile's per-
# engine stream is a projection of one topological order, so a producer's
# trigger always precedes a consumer's wait and engine-level blocking cannot
# deadlock; sequential waits on monotonic sems == simultaneous waits.
# ---------------------------------------------------------------------------
_SPLIT_ID = [0]


def _split_multi_waits(nc):
    for f in nc.m.functions:
        for blk in f.blocks:
            insts = blk.instructions
            for idx in range(len(insts) - 1, -1, -1):
                inst = insts[idx]
                si = inst.sync_info
                waits = list(si.on_wait or []) if si is not None else []
                if len(waits) <= 1:
                    continue
                # For DMA instructions, keep a COMPUTE dependency on the
                # instruction (it rides the queue descriptor, so the DMA
                # pipeline pre-runs while parked on the sem) and hoist the
                # early-firing queue-guard sems onto the engine nop.
                if type(inst).__name__ == "InstDMACopy":
                    comp = [
                        w
                        for w in waits
                        if not str(w.ant_name or "").startswith("DMA")
                    ]
                    if comp:
                        keep = comp[-1]
                        waits = [w for w in waits if w is not keep] + [keep]
                inst.sync_info = mybir.SyncInfo(
                    on_wait=[waits[-1]], on_update=list(si.on_update or [])
                )
                for w in reversed(waits[:-1]):
                    _SPLIT_ID[0] += 1
                    nop = mybir.InstNoOp(
                        name=f"I-waitsplit-{_SPLIT_ID[0]}", ins=[], outs=[]
                    )
                    nop.engine = inst.engine
                    nop.sync_info = mybir.SyncInfo(on_wait=[w], on_update=[])
                    insts.insert(idx, nop)


def _rewrite_range_clears(nc):
    """This walrus build rejects the EVENT_SEMAPHORE_RANGE_CLEAR raw-ISA
    encoding ("ISA wrong length"); replace each with per-sem
    InstEventSemaphore sem-wr-imm 0 writes on the same engine."""
    for f in nc.m.functions:
        for blk in f.blocks:
            insts = blk.instructions
            for idx in range(len(insts) - 1, -1, -1):
                inst = insts[idx]
                if type(inst).__name__ != "InstISA":
                    continue
                s = str(inst)
                if "EVENT_SEMAPHORE_RANGE_CLEAR" not in s:
                    continue
                import re

                first = int(re.search(r"range_first=(\d+)", s).group(1))
                last = int(re.search(r"range_last=(\d+)", s).group(1))
                si = inst.sync_info
                waits = list(si.on_wait or []) if si is not None else []
                upds = list(si.on_update or []) if si is not None else []
                repl = []
                for j, sem in enumerate(range(first, last + 1)):
                    _SPLIT_ID[0] += 1
                    ev = mybir.InstEventSemaphore(
                        name=f"I-semclr-{_SPLIT_ID[0]}", ins=[], outs=[]
                    )
                    ev.engine = inst.engine
                    ev.sync_info = mybir.SyncInfo(
                        on_wait=waits if j == 0 else [],
                        on_update=[
                            mybir.SyncUpdate(
                                sync_type="semaphore",
                                id=sem,
                                update_mode="sem-wr-imm",
                                update_value=0,
                            )
                        ]
                        + (upds if j == (last - first) else []),
                    )
                    repl.append(ev)
                insts[idx : idx + 1] = repl


def _trim_tail(nc):
    """Exec time ends when the last engine halts.  The TileContext tail is
    [drain+barrier, 20 serial sem-clears on Pool, second barrier] -- ~2.5us
    after the output DMA completes.  Re-execution of the NEFF only needs the
    sems cleared before the tile block runs, so: clear them in the MAIN
    block instead (spread across engines, before the existing all-engine
    barrier that already orders engine start), and delete the tail clears +
    second barrier."""
    f = nc.m.functions[0]
    blocks = {b.name: b for b in f.blocks}
    main = blocks["main"]
    end = [b for n, b in blocks.items() if n.endswith("_end")][0]

    insts = end.instructions
    # find the Pool drain that precedes the semclear run (after barrier-1)
    clr_idx = [i for i, x in enumerate(insts) if x.name.startswith("I-semclr-")]
    if not clr_idx:
        return
    first, last = clr_idx[0], clr_idx[-1]
    clears = insts[first : last + 1]
    # everything after the clears is barrier-2 (+ its drains): delete; also
    # delete the clears and the extra Pool drain right before them
    start_del = first
    if start_del > 0 and type(insts[start_del - 1]).__name__ == "InstDrain":
        start_del -= 1
    del insts[start_del:]

    # re-insert clears near the start of main, round-robin across engines,
    # before the all-engine barrier (the barrier orders them vs tile work)
    m_insts = main.instructions
    # insertion point: before the first InstDrain (start of the barrier)
    ins_pt = next(
        (i for i, x in enumerate(m_insts) if type(x).__name__ == "InstDrain"),
        len(m_insts),
    )
    engines = [
        mybir.EngineType.Pool,
        mybir.EngineType.DVE,
        mybir.EngineType.Activation,
        mybir.EngineType.PE,
        mybir.EngineType.SP,
    ]
    for j, c in enumerate(clears):
        c.engine = engines[j % len(engines)]
        c.sync_info = mybir.SyncInfo(
            on_wait=[], on_update=list(c.sync_info.on_update or [])[:1]
        )
        m_insts.insert(ins_pt + j, c)

# ---------------------------------------------------------------------------

B, D, NCLS, KC = 8192, 512, 90, 8
NCORES, P = 8, 128
BC = B // NCORES          # samples per core
KCH = D // P              # contraction chunks
NTILE = 512               # moving-operand columns per matmul (fp32 max)
NH = BC // NTILE          # halves per core (2)

_BUILD_CACHE = {}


def _build(nslot, post_process=True):
    M8 = 8 * nslot            # center-score rows
    M9 = 9 * nslot            # + u-candidate rows
    assert M9 <= 128, f"class slots {nslot} need {M9} > 128 partitions"
    NB = NTILE // P           # 128-sample blocks per half
    f32 = mybir.dt.float32
    bf16 = mybir.dt.bfloat16
    nc = bass.Bass("TRN2", target_bir_lowering=False, debug=False, num_devices=1)
    # xt is pre-chunked on the host: chunk (h, k) is a contiguous
    # [128, 512] block, so each chunk DMA is one linear 256 KiB read.
    xt_d = nc.dram_tensor("xt", [NH, KCH, P, NTILE], f32, kind="ExternalInput")
    # partition-major: [128, KCH, 128] so each partition row is one
    # contiguous 2 KiB read
    ct_d = nc.dram_tensor("ct", [P, KCH, P], f32, kind="ExternalInput")
    e_d = nc.dram_tensor("e", [nslot, NH, NTILE], bf16, kind="ExternalInput")
    g_d = nc.dram_tensor("g", [nslot, P], bf16, kind="ExternalInput")
    # vb: per-partition (scale, bias) for the ACT assembly op
    vb_d = nc.dram_tensor("vb", [P, 2], f32, kind="ExternalInput")
    sel_d = nc.dram_tensor("sel", [P, P], f32, kind="ExternalInput")
    val_d = nc.dram_tensor("val", [P, NH * NB], f32, kind="ExternalOutput")

    with tile.TileContext(nc) as tc:
        with ExitStack() as ctx:
            consts = ctx.enter_context(tc.tile_pool(name="consts", bufs=1))
            work = ctx.enter_context(tc.tile_pool(name="work", bufs=2))
            pwu = ctx.enter_context(tc.tile_pool(name="pwu", bufs=1, space="PSUM"))
            pst = ctx.enter_context(tc.tile_pool(name="pst", bufs=2, space="PSUM"))
            pmk = ctx.enter_context(tc.tile_pool(name="pmk", bufs=2, space="PSUM"))
            puq = ctx.enter_context(tc.tile_pool(name="puq", bufs=2, space="PSUM"))
            ptr = ctx.enter_context(tc.tile_pool(name="ptr", bufs=1, space="PSUM"))

            # warm-up zeros tile: no input dependency, so the PE can start
            # ramping its clock immediately after the gpsimd memset.
            wz = consts.tile([P, NTILE], f32)
            nc.gpsimd.memset(wz, 0.0)

            # input DMAs, spread across the 3 queue-issuing engines so issue
            # cost (~0.7us per DMA_DIRECT2D) overlaps; order within each
            # engine = order the matmuls will need the data.
            xt_sb = consts.tile([P, KCH, BC], f32)
            ct_sb = consts.tile([P, KCH, P], f32)
            e_sb = consts.tile([nslot, NH, NTILE], bf16)
            g_sb = consts.tile([nslot, P], bf16)
            vb_sb = consts.tile([P, 2], f32)
            sel_sb = consts.tile([P, P], f32)
            xt_ap = xt_d.ap()

            nc.gpsimd.dma_start(out=e_sb, in_=e_d.ap())
            nc.gpsimd.dma_start(out=g_sb, in_=g_d.ap())
            nc.gpsimd.dma_start(out=vb_sb, in_=vb_d.ap())
            nc.gpsimd.dma_start(out=sel_sb, in_=sel_d.ap())
            nc.scalar.dma_start(out=ct_sb, in_=ct_d.ap())

            xt_engines = {
                (0, 0): nc.sync, (0, 1): nc.scalar,
                (0, 2): nc.sync, (0, 3): nc.scalar,
                (1, 0): nc.sync, (1, 1): nc.scalar,
                (1, 2): nc.sync, (1, 3): nc.sync,
            }
            for h in range(NH):
                for k in range(KCH):
                    xt_engines[(h, k)].dma_start(
                        out=xt_sb[:, k, h * NTILE : (h + 1) * NTILE],
                        in_=xt_ap[h, k],
                    )

            ident = consts.tile([P, P], f32)
            make_identity(nc, ident)

            # PE warm-up: released by the wz memset only, runs while the
            # first xt chunks are in flight (HAM clock-gate 1.2 -> 2.4 GHz).
            wu_ps = pwu.tile([P, NTILE], f32)
            for w in range(4):
                nc.tensor.matmul(
                    wu_ps, wz[:, 0:P], wz, start=True, stop=True,
                    skip_group_check=True,
                )

            # mask expansion for both halves (tiny bf16 matmuls; also keeps
            # the PE busy while xt lands).  mask[r, b] = 1 iff row r belongs
            # to sample b's class slot (8 center rows + 1 csum row).
            mask_sb = []
            for h in range(NH):
                mk_ps = pmk.tile([P, NTILE], f32)
                nc.tensor.matmul(
                    mk_ps, g_sb, e_sb[:, h, :], start=True, stop=True
                )
                mk = work.tile([P, NTILE], f32)
                nc.scalar.copy(mk, mk_ps)
                mask_sb.append(mk)

            # main fp32 matmuls: S = table^T @ x^T, [128, 512] per half
            st_ps = []
            for h in range(NH):
                sp = pst.tile([P, NTILE], f32)
                for k in range(KCH):
                    nc.tensor.matmul(
                        sp,
                        ct_sb[:, k, :],
                        xt_sb[:, k, h * NTILE : (h + 1) * NTILE],
                        start=(k == 0),
                        stop=(k == KCH - 1),
                    )
                st_ps.append(sp)

            # epilogue per half:
            #   v    = S*scalevec + biasvec   (ACT: S on sq rows, 1 on u rows)
            #   tmp  = S * mask               (DVE)
            #   t    = tmp * v                (DVE: s^2*mask | u*mask | 0)
            #   uq   = sel^T @ t              (PE fp32: row0 = q+2u, row1 = u)
            # then per-128-block PE transposes put (num-8, den-8) into
            # per-sample partitions, and one short DVE chain finishes.
            tr_ps = ptr.tile([P, NH * NB * 2], f32)
            uq_sbs = []
            for h in range(NH):
                sp = st_ps[h]
                v = work.tile([P, NTILE], f32)
                nc.scalar.activation(
                    v, sp, mybir.ActivationFunctionType.Identity,
                    bias=vb_sb[:, 1:2], scale=vb_sb[:, 0:1],
                )
                tmp = work.tile([P, NTILE], f32)
                nc.vector.tensor_mul(tmp, sp, mask_sb[h])
                t = work.tile([P, NTILE], f32)
                nc.vector.tensor_mul(t, tmp, v)

                uq_ps = puq.tile([P, NTILE], f32)
                nc.tensor.matmul(uq_ps, sel_sb, t, start=True, stop=True)
                uq_sb = work.tile([2, NTILE], f32)
                nc.scalar.copy(uq_sb, uq_ps[0:2])
                uq_sbs.append(uq_sb)
                for j in range(NB):
                    nc.tensor.matmul(
                        tr_ps[:, (h * NB + j) * 2 : (h * NB + j) * 2 + 2],
                        uq_sb[:, j * P : (j + 1) * P],
                        ident[0:2, 0:2],
                        is_transpose=True,
                        start=True,
                        stop=True,
                        skip_group_check=True,
                    )

            # final chain on [128, NH*NB] tiles (num = col0+8, den = col1+8)
            tr4 = tr_ps.rearrange("p (c two) -> p c two", two=2)
            num = consts.tile([P, NH * NB], f32)
            nc.vector.tensor_scalar_add(num, tr4[:, :, 0], 8.0)
            den = consts.tile([P, NH * NB], f32)
            nc.vector.tensor_scalar_add(den, tr4[:, :, 1], 8.0)
            rden = consts.tile([P, NH * NB], f32)
            nc.vector.reciprocal(rden, den)
            val_sb = consts.tile([P, NH * NB], f32)
            nc.vector.tensor_mul(val_sb, num, rden)
            nc.sync.dma_start(out=val_d.ap(), in_=val_sb)
    if post_process:
        _rewrite_range_clears(nc)
        _trim_tail(nc)
        _split_multi_waits(nc)
    return nc


def _prep_in_maps(x, centers, labels):
    import ml_dtypes

    x = np.ascontiguousarray(np.asarray(x, dtype=np.float32))
    centers = np.asarray(centers, dtype=np.float32)
    labels = np.asarray(labels).astype(np.int64)
    order = np.argsort(labels, kind="stable")
    xs = x[order]
    ls = labels[order]

    core_classes = [np.unique(ls[i * BC : (i + 1) * BC]) for i in range(NCORES)]
    nslot = max(len(c) for c in core_classes)
    M8, M9 = 8 * nslot, 9 * nslot

    # sel col0: num-8 = 1*q + 2*u ; col1: den-8 = 1*u  (cols 2.. zero)
    sel = np.zeros((P, P), np.float32)
    sel[:M8, 0] = 1.0
    sel[M8:M9, 0] = 2.0
    sel[M8:M9, 1] = 1.0
    # vb: (scale, bias) rows for the ACT assembly: v = S on sq rows, 1.0 on
    # u rows, 0 elsewhere
    vb = np.zeros((P, 2), np.float32)
    vb[:M8, 0] = 1.0
    vb[M8:M9, 1] = 1.0
    # g: slot -> row-mask expansion pattern (8 center rows + 1 csum row)
    g = np.zeros((nslot, P), np.float32)
    for s in range(nslot):
        g[s, 8 * s : 8 * s + 8] = 1.0
        g[s, M8 + s] = 1.0
    g = g.astype(ml_dtypes.bfloat16)

    in_maps = []
    for i in range(NCORES):
        sl = slice(i * BC, (i + 1) * BC)
        # chunk-contiguous layout [h, k, 128, 512] (see _build)
        xT = np.ascontiguousarray(
            xs[sl].T.reshape(KCH, P, NH, NTILE).transpose(2, 0, 1, 3)
        )
        cls = core_classes[i]
        ct = np.zeros((D, P), np.float32)
        for j, c in enumerate(cls):
            ct[:, 8 * j : 8 * j + 8] = centers[c].T
            ct[:, M8 + j] = centers[c].sum(axis=0)
        ct = np.ascontiguousarray(ct.reshape(KCH, P, P).transpose(1, 0, 2))
        slot_of = {c: j for j, c in enumerate(cls)}
        slots = np.array([slot_of[c] for c in ls[sl]])
        e = np.zeros((nslot, BC), np.float32)
        e[slots, np.arange(BC)] = 1.0
        e = np.ascontiguousarray(e.reshape(nslot, NH, NTILE)).astype(
            ml_dtypes.bfloat16
        )
        in_maps.append({"xt": xT, "ct": ct, "e": e, "g": g, "vb": vb, "sel": sel})
    return nslot, in_maps


def kernel(x, centers, labels, _trace=False):
    nslot, in_maps = _prep_in_maps(x, centers, labels)
    if nslot not in _BUILD_CACHE:
        _BUILD_CACHE[nslot] = _build(nslot)
    nc = _BUILD_CACHE[nslot]
    res = bass_utils.run_bass_kernel_spmd(
        nc, in_maps, core_ids=list(range(NCORES)), trace=_trace
    )
    total = 0.0
    for r in res.results:
        total += r["val"].astype(np.float64).sum()
    out = np.float32(total / B)
    if _trace:
        return out, res
    return out
